# revision 13
# baseline (speedup 1.0000x reference)
"""Associative-embedding (AE) loss kernel for Trainium2, 8 NeuronCores.

Problem: tags [32, 262144, 1] f32, keypoints [32, 30, 17, 2] int
(col0 = flat heatmap index, col1 = valid flag). Output [32, 2] f32 =
stack([pull, push], axis=1) per batch.

Strategy (pure data parallel, 4 batches per core), v2:
  - Host maps each (person, joint) slot to the 256B-aligned row of the
    tags shard holding its value (row = flat>>6) and builds a one-hot
    lane-selection mask with the valid/cnt weight folded in
    (wm[p,j,lane] = w if lane == flat&63). 120 real persons, 8 pad.
  - Device issues THREE dma_gathers (1024+1024+128 row descriptors,
    ~994ns fixed + 0.34ns/desc SWDGE cost each; the Q7 descriptor ring
    holds at most 1024) instead of 12+ chained 128-wide indirect DMAs
    whose per-instruction fixed cost dominated the old kernel. (The
    InstDMACopy indirect path only consumes one offset per partition,
    so it cannot batch scattered singles.) Two SWDGE queues avoid
    descriptor-ring stalls between the 1024-deep gathers.
  - Per gather slice, two DVE scalar_tensor_tensor passes with
    accum_out fuse the lane extraction, weighting, squaring and the
    joint+lane reduction (mean = sum(rows*wm), m2 = sum((rows*wm)*rows),
    all f32), pipelined behind the remaining gathers; partial sums are
    combined with one tensor_scalar per moment.
  - One PE transpose (bf16 identity) yields the mean row; the pairwise
    exp argument is built by TWO accumulating matmuls: rank-1 mean x
    mean plus rank-6 [ones, ones, e_b..] x [-m^2/2, -PEN_OUT/2,
    r*e_b..], with the -m_p^2 term injected via the scalar engine's
    per-partition activation bias. bf16 rounding of the penalty
    constants is compensated exactly on the host (S_EFF / PSCALE2).
  - scalar Exp (scale=2) with free-axis accum gives per-person pull
    sums; DVE removes the diagonal and applies the pull scale; a single
    [128,4]^T @ [128,2] matmul reduces persons -> per-batch [pull,push].
  - All matmul inputs bf16 (single PE pass vs fp32's LOW+HIGH double
    pass); PSUM accumulation and the mean/m2 statistics stay f32.

Each core returns its own [4, 2] rows; the host concatenates to [32, 2].
"""

import os
import sys

import numpy as np

if "/opt/trn_rl_repo" not in sys.path:
    sys.path.insert(0, "/opt/trn_rl_repo")

import ml_dtypes

import concourse.bacc as bacc
import concourse.bass as bass
import concourse.tile as tile
from concourse import mybir
from concourse.bass_utils import run_bass_kernel_spmd

# Problem constants (hardcoded per the harness contract)
B, N, D = 32, 262144, 1
P, J = 30, 17
NCORES = 8
BL = B // NCORES          # 4 local batches per core
NFLAT = BL * N            # 1048576 f32 elements in the per-core tags shard
PP = 128                  # person slots (120 real + 8 pad)
NPER = BL * P             # 120 persons per core
PULL_SCALE = 0.5 / (P * (P - 1) / 2.0) * 0.5      # 1/1740
PEN_IN = -float(np.log(PULL_SCALE))               # ~7.46, same-batch offdiag
PEN_OUT = 60.0                                    # exp(-60) == 0 in f32

_F32 = mybir.dt.float32
_I32 = mybir.dt.int32
_BF16 = mybir.dt.bfloat16

# bf16-rounded penalty constants actually seen by the PE, and the exact
# host-side compensation so the final pull scale is unaffected.
_R_BF = float(np.asarray((PEN_OUT - PEN_IN) / 2.0, ml_dtypes.bfloat16))
_C_BF = float(np.asarray(-PEN_OUT / 2.0, ml_dtypes.bfloat16))  # -30, exact
PEN_IN_EFF = -2.0 * (_C_BF + _R_BF)
S_EFF = float(np.exp(-PEN_IN_EFF))      # diagonal exp value to subtract
PSCALE2 = PULL_SCALE / S_EFF            # rescale so same-batch scale is exact


def _build_bass():
    nc = bacc.Bacc("TRN2", target_bir_lowering=False, debug=False,
                   num_devices=NCORES, num_swdge_queues=2)

    tags_ext = nc.dram_tensor("tags", [NFLAT // 64, 64], _F32,
                              kind="ExternalInput")
    idxs_ext = nc.dram_tensor("gidx", [PP, (PP * J) // 16], mybir.dt.int16,
                              kind="ExternalInput")
    wm_ext = nc.dram_tensor("wmask", [PP, J, 64], _F32, kind="ExternalInput")
    hl_ext = nc.dram_tensor("hlmat", [6, PP], _BF16, kind="ExternalInput")
    hr_ext = nc.dram_tensor("hrmat", [6, PP], _BF16, kind="ExternalInput")
    ws_ext = nc.dram_tensor("wsel", [PP, BL], _BF16, kind="ExternalInput")
    id_ext = nc.dram_tensor("ident", [PP, PP], _BF16, kind="ExternalInput")
    out_ext = nc.dram_tensor("out", [BL, 2], _F32, kind="ExternalOutput")

    with tile.TileContext(nc) as tc:
        with tc.tile_pool(name="sb", bufs=1) as pool, \
             tc.tile_pool(name="ps", bufs=1, space="PSUM") as psum:
            # Uploads, split across the two HWDGE queues; gidx first (it
            # gates the gather).
            idxs_t = pool.tile([PP, (PP * J) // 16], mybir.dt.int16)
            nc.sync.dma_start(idxs_t[:], idxs_ext[:])
            wm_t = pool.tile([PP, J, 64], _F32)
            nc.scalar.dma_start(wm_t[:], wm_ext[:])
            hl_t = pool.tile([6, PP], _BF16)
            nc.sync.dma_start(hl_t[:], hl_ext[:])
            hr_t = pool.tile([6, PP], _BF16)
            nc.scalar.dma_start(hr_t[:], hr_ext[:])
            ws_t = pool.tile([PP, BL], _BF16)
            nc.sync.dma_start(ws_t[:], ws_ext[:])
            id_t = pool.tile([PP, PP], _BF16)
            nc.sync.dma_start(id_t[:], id_ext[:])

            # Warm the scalar engine's Exp table during the gather window.
            zdum = pool.tile([PP, 1], _F32)
            nc.vector.memset(zdum[:], 0.0)
            edum = pool.tile([PP, 1], _F32)
            nc.scalar.activation(edum[:], zdum[:],
                                 mybir.ActivationFunctionType.Exp)

            # Gathers: rows[p, j, :] = tags row (flat>>6) of slot (p, j),
            # split j into [0:8), [8:16), [16:17) to fit the 1024-entry
            # descriptor ring. Per slice, fused lane-select + weight +
            # reduce: t1 = rows*wm -> partial mean; t2 = t1*rows -> m2.
            slices = [(0, 8, 0), (8, 16, 1), (16, J, 0)]
            means, m2s = [], []
            for j0, j1, qn in slices:
                nj = j1 - j0
                rows_s = pool.tile([PP, nj, 64], _F32, name=f"rows_{j0}")
                nc.gpsimd.dma_gather(
                    out_ap=rows_s[:, :, :], in_ap=tags_ext[:],
                    idxs_ap=idxs_t[:, (j0 * PP) // 16:(j1 * PP) // 16],
                    num_idxs=PP * nj, num_idxs_reg=PP * nj,
                    elem_size=64, queue_num=qn,
                )
                t1 = pool.tile([PP, nj, 64], _F32, name=f"t1_{j0}")
                mean_s = pool.tile([PP, 1], _F32, name=f"mean_{j0}")
                nc.vector.scalar_tensor_tensor(
                    out=t1[:, :, :], in0=rows_s[:, :, :], scalar=1.0,
                    in1=wm_t[:, j0:j1, :],
                    op0=mybir.AluOpType.mult, op1=mybir.AluOpType.mult,
                    accum_out=mean_s[:],
                )
                t2 = pool.tile([PP, nj, 64], _F32, name=f"t2_{j0}")
                m2_s = pool.tile([PP, 1], _F32, name=f"m2_{j0}")
                nc.vector.scalar_tensor_tensor(
                    out=t2[:, :, :], in0=t1[:, :, :], scalar=1.0,
                    in1=rows_s[:, :, :],
                    op0=mybir.AluOpType.mult, op1=mybir.AluOpType.mult,
                    accum_out=m2_s[:],
                )
                means.append(mean_s)
                m2s.append(m2_s)

            mean_c = pool.tile([PP, 1], _F32)
            nc.vector.tensor_scalar(
                out=mean_c[:], in0=means[0][:], scalar1=means[1][:],
                scalar2=means[2][:],
                op0=mybir.AluOpType.add, op1=mybir.AluOpType.add,
            )
            m2_c = pool.tile([PP, 1], _F32)
            nc.vector.tensor_scalar(
                out=m2_c[:], in0=m2s[0][:], scalar1=m2s[1][:],
                scalar2=m2s[2][:],
                op0=mybir.AluOpType.add, op1=mybir.AluOpType.add,
            )

            # bf16 mean column -> transpose -> mean row (all later users of
            # the mean see the same bf16-rounded value for consistency)
            mean_bf = pool.tile([PP, 1], _BF16)
            nc.vector.tensor_copy(mean_bf[:], mean_c[:])
            mrow_ps = psum.tile([1, PP], _BF16)
            nc.tensor.transpose(mrow_ps[:], mean_bf[:], id_t[:])
            mr_sb = pool.tile([1, PP], _BF16)
            nc.vector.tensor_copy(mr_sb[:], mrow_ps[:])
            # device row of Hr: zc[q] = -mean[q]^2/2
            nc.vector.scalar_tensor_tensor(
                out=hr_t[0:1, :], in0=mr_sb[:], scalar=-0.5, in1=mr_sb[:],
                op0=mybir.AluOpType.mult, op1=mybir.AluOpType.mult,
            )

            # exp bias: -mean[p]^2 (from the same bf16-rounded mean)
            negmsq = pool.tile([PP, 1], _F32)
            nc.vector.scalar_tensor_tensor(
                out=negmsq[:], in0=mean_bf[:], scalar=-1.0, in1=mean_bf[:],
                op0=mybir.AluOpType.mult, op1=mybir.AluOpType.mult,
            )

            # push column: X[:,1] = (m2 - mean^2) / P  (pure f32 path)
            x_t = pool.tile([PP, 2], _BF16)
            sqr = pool.tile([PP, 1], _F32)
            nc.vector.scalar_tensor_tensor(
                out=sqr[:], in0=mean_c[:], scalar=-1.0 / P, in1=mean_c[:],
                op0=mybir.AluOpType.mult, op1=mybir.AluOpType.mult,
            )
            nc.vector.scalar_tensor_tensor(
                out=x_t[:, 1:2], in0=m2_c[:], scalar=1.0 / P, in1=sqr[:],
                op0=mybir.AluOpType.mult, op1=mybir.AluOpType.add,
            )

            # Z[p,q] = mp*mq - mq^2/2 - PEN_OUT/2 + r*same(p,q); the -mp^2/2
            # half enters through the exp bias.
            z_ps = psum.tile([PP, PP], _F32)
            nc.tensor.matmul(z_ps[:], mr_sb[:], mr_sb[:], start=True,
                             stop=False, skip_group_check=True)
            nc.tensor.matmul(z_ps[:], hl_t[:], hr_t[:], start=False,
                             stop=True, skip_group_check=True)

            # exp(2Z - mp^2) with free-axis accumulation -> per-person pull
            e_t = pool.tile([PP, PP], _BF16)
            x0 = pool.tile([PP, 1], _F32)
            nc.scalar.activation(e_t[:], z_ps[:],
                                 mybir.ActivationFunctionType.Exp, scale=2.0,
                                 bias=negmsq[:], accum_out=x0[:])

            # pull column: drop the diagonal exp(-PEN_IN_EFF), apply scale
            nc.vector.tensor_scalar(
                out=x_t[:, 0:1], in0=x0[:], scalar1=S_EFF, scalar2=PSCALE2,
                op0=mybir.AluOpType.subtract, op1=mybir.AluOpType.mult,
            )

            # persons -> batches: single [128,4]^T @ [128,2] matmul
            out_ps = psum.tile([BL, 2], _F32)
            nc.tensor.matmul(out_ps[:], ws_t[:], x_t[:], start=True,
                             stop=True, skip_group_check=True)

            res = pool.tile([BL, 2], _F32)
            nc.vector.tensor_copy(res[:], out_ps[:])
            nc.sync.dma_start(out_ext[:], res[:])

    nc.compile()
    return nc


def _prep_core_inputs(core: int, tags: np.ndarray, kp: np.ndarray) -> dict:
    """Host-side preprocessing: shard + per-person index/mask tables."""
    b0 = core * BL
    t = np.ascontiguousarray(
        tags[b0:b0 + BL].reshape(NFLAT, 1).astype(np.float32, copy=False))

    idx = kp[b0:b0 + BL, :, :, 0].astype(np.int64)       # [BL,P,J]
    val = (kp[b0:b0 + BL, :, :, 1] == 1)                 # [BL,P,J]
    cnt = np.maximum(val.sum(-1), 1).astype(np.float32)  # [BL,P]

    flat = (idx + np.arange(BL)[:, None, None] * N).reshape(NPER, J)
    valf = val.reshape(NPER, J)

    rowq = np.zeros((PP, J), np.int16)
    rowq[:NPER] = (flat >> 6).astype(np.int16)
    lane = np.zeros((PP, J), np.int64)
    lane[:NPER] = flat & 63

    # index i = p + 128*j lives at idxs[i % 16, i // 16]; the 16-partition
    # block is replicated across all 128 partitions for the 8 Q7 cores.
    i = np.arange(PP * J)
    idxs16 = np.zeros((16, (PP * J) // 16), np.int16)
    idxs16[i % 16, i // 16] = rowq[i % PP, i // PP]
    idxs = np.tile(idxs16, (PP // 16, 1))

    wm = np.zeros((PP, J, 64), np.float32)
    w = np.zeros((PP, J), np.float32)
    w[:NPER] = valf / cnt.reshape(NPER)[:, None]
    pg, jg = np.meshgrid(np.arange(PP), np.arange(J), indexing="ij")
    wm[pg, jg, lane] = w

    ebs = np.zeros((BL, PP), np.float32)
    for b in range(BL):
        ebs[b, b * P:(b + 1) * P] = 1.0
    hl = np.zeros((6, PP), np.float32)
    hl[0, :] = 1.0                 # pairs the device zc row
    hl[1, :] = 1.0                 # pairs the constant row
    hl[2:6] = ebs
    hr = np.zeros((6, PP), np.float32)
    hr[1, :] = -PEN_OUT / 2.0
    hr[2:6] = _R_BF * ebs

    ident = np.eye(PP, dtype=np.float32)

    return {"tags": t.reshape(NFLAT // 64, 64), "gidx": idxs, "wmask": wm,
            "hlmat": hl.astype(ml_dtypes.bfloat16),
            "hrmat": hr.astype(ml_dtypes.bfloat16),
            "wsel": np.ascontiguousarray(ebs.T).astype(ml_dtypes.bfloat16),
            "ident": ident.astype(ml_dtypes.bfloat16)}


_NC_CACHE = {}


def _get_nc():
    if "nc" not in _NC_CACHE:
        _NC_CACHE["nc"] = _build_bass()
    return _NC_CACHE["nc"]


def _ensure_profile_hook():
    """Provide antenv.axon_hooks if the image's antenv lacks it, so
    run_bass_kernel_spmd(trace=True) can capture NTFF profiles under axon.
    Mirrors trn_agent_boot's ctypes shim over libaxon_pjrt.so."""
    try:
        from antenv.axon_hooks import get_axon_ntff_profile_hook  # noqa: F401
        return
    except ImportError:
        pass
    import contextlib
    import ctypes
    import types

    so_path = "/opt/axon/libaxon_pjrt.so"
    if not os.path.exists(so_path):
        return
    lib = ctypes.CDLL(so_path)
    if not hasattr(lib, "axon_start_nrt_profile"):
        return
    lib.axon_start_nrt_profile.argtypes = [ctypes.POINTER(ctypes.c_int64),
                                           ctypes.c_size_t]
    lib.axon_start_nrt_profile.restype = ctypes.c_int64
    lib.axon_stop_nrt_profile.argtypes = [ctypes.c_char_p]
    lib.axon_stop_nrt_profile.restype = ctypes.c_int64

    @contextlib.contextmanager
    def _hook(output_dir, device_ids):
        import jax
        jax.devices()
        if device_ids:
            ids = (ctypes.c_int64 * len(device_ids))(*device_ids)
            rc = lib.axon_start_nrt_profile(ids, len(device_ids))
        else:
            rc = lib.axon_start_nrt_profile(None, 0)
        if rc != 0:
            raise RuntimeError(f"axon_start_nrt_profile rc={rc}")
        try:
            yield
        finally:
            n = lib.axon_stop_nrt_profile(str(output_dir).encode())
            print(f"profile: {n} file(s) written to {output_dir}",
                  file=sys.stderr)

    mod = types.ModuleType("antenv.axon_hooks")
    _state = {"hook": _hook}
    mod.set_axon_ntff_profile_hook = lambda h: _state.__setitem__("hook", h)
    mod.get_axon_ntff_profile_hook = lambda: _state["hook"]
    sys.modules["antenv.axon_hooks"] = mod


def run(tags: np.ndarray, keypoints: np.ndarray, **spmd_kwargs):
    """Build in_maps, run on 8 cores, return ([32,2] f32, BassKernelResults)."""
    tags = np.asarray(tags)
    kp = np.asarray(keypoints)
    if spmd_kwargs.get("trace"):
        _ensure_profile_hook()
    nc = _get_nc()
    in_maps = [_prep_core_inputs(c, tags, kp) for c in range(NCORES)]
    results = run_bass_kernel_spmd(nc, in_maps, core_ids=list(range(NCORES)),
                                   **spmd_kwargs)
    out = np.concatenate([np.asarray(results.results[c]["out"])
                          for c in range(NCORES)], axis=0)
    return out.astype(np.float32), results


def kernel(tags: np.ndarray, keypoints: np.ndarray) -> np.ndarray:
    out, _ = run(tags, keypoints)
    return out
